# revision 11
# baseline (speedup 1.0000x reference)
"""2-layer GIN + attentional pooling on 8 Trainium2 NeuronCores (Bass/Tile).

v2 (correctness + speed over the original baseline):
  - L1 edge gather is done on the HOST: the windowed per-(block, chunk)
    edge-value stream xs (f16) is pre-gathered from x and DMA-streamed, so
    L1 uses no GPSIMD edge ap_gather and no 16MB SBUF x-table. The
    cumsum + ends-gather + diff machinery is unchanged.
  - L2 unchanged: SBUF table2 built from AllGather'd h1 (f16 pairs),
    GPSIMD ap_gather per src-block group, DVE cumsum, ends-gather, diff,
    PE fold.
  - POOL rewritten for exactness: per-graph segment sums via PE one-hot
    matmuls (transpose each 128-node block of [ee; ee*t] with eye32/e33,
    then accumulate into a persistent PSUM slot grid with host-built
    mini-S selectors). The old cross-graph running-cumsum difference had
    catastrophic f32 cancellation (per-graph gate maxima span e^16).
    Constant-shift exp stays: per-graph sums are now exact zeros across
    graphs, so only within-graph f32 rounding remains (~1e-4).
"""
import os
import sys

os.environ.setdefault("NEURON_RT_RESET_CORES", "1")
sys.path.insert(0, '/opt/trn_rl_repo')

import numpy as np

# -- NTFF profiling hook shim (optional; enables trace=True under axon) ----
def _install_ntff_shim():
    import types
    try:
        import antenv
        if 'antenv.axon_hooks' in sys.modules:
            return
        hooks = types.ModuleType('antenv.axon_hooks')
        _state = {'hook': None}
        hooks.set_axon_ntff_profile_hook = lambda h: _state.__setitem__('hook', h)
        hooks.get_axon_ntff_profile_hook = lambda: _state['hook']
        sys.modules['antenv.axon_hooks'] = hooks
        antenv.axon_hooks = hooks
        from trn_agent_boot.trn_boot import _ntff_profile_via_ctypes
        h = _ntff_profile_via_ctypes('/opt/axon/libaxon_pjrt.so')
        if h is not None:
            hooks.set_axon_ntff_profile_hook(h)
    except Exception:
        pass


_install_ntff_shim()

N_NODES = 262144
N_GRAPHS = 1024
C_IN = 16
H = 32
NC = 8
BLK = 32768
NCH1, ECH1, NCHUNK1 = 2112, 4608, 16
NCH2, ECH2, NCHUNK2 = 1056, 2432, 32
NMAX = NCH1 * NCHUNK1            # 33792
SOFTMAX_SHIFT = 20.0
MAX_WAITS = 1
TILE_N = 512
GW2 = 264                        # pool slot grid (max graphs/core + margin)
SW = 24                          # mini-S slot window per 128-node block
NBLKP = NMAX // 128              # 264 pool node blocks


def _pool_off(b):
    """Core-independent psum slot offset for pool node-block b."""
    return max(0, min(b // 2 - 8, GW2 - SW))

_cache = {}


def _split_multi_waits(nc, mybir, max_waits=MAX_WAITS):
    n_split = 0
    for fn in nc.m.functions:
        for bb in fn.blocks:
            out = []
            for ins in bb.instructions:
                si = ins.sync_info
                if si is not None and si.on_wait and len(si.on_wait) > max_waits:
                    waits = list(si.on_wait)
                    extra = waits[:-max_waits]
                    keep = waits[-max_waits:]
                    for i in range(0, len(extra), max_waits):
                        group = extra[i:i + max_waits]
                        nop = mybir.InstNoOp(
                            name=f"waitsplit_{nc.next_id()}",
                            sync_info=mybir.SyncInfo(on_wait=group, on_update=[]),
                            bass_nofuse=True,
                            engine=ins.engine,
                        )
                        out.append(nop)
                        n_split += 1
                    si.on_wait = keep
                out.append(ins)
            bb.instructions = out
    return n_split


def _wrap_idx(vals, group, arr, col0=0):
    """Wrapped ap_gather index layout: value i -> arr[16g + i%16, col0 + i//16]."""
    n = len(vals)
    assert n % 16 == 0
    v = np.asarray(vals, dtype=np.int16).reshape(n // 16, 16).T
    arr[16 * group:16 * group + 16, col0:col0 + n // 16] = v


def _register_cumsum():
    from concourse import dve_ops
    from concourse.dve_spec import Spec, Src0, C0, AluOp, lower
    import concourse.dve_spec as ds
    from concourse.dve_uop import DveOpSpec
    for op in dve_ops.OPS:
        if op.name == "CUMSUM_ANT":
            return op
    spec = Spec(
        body=ds.scan(AluOp.ADD, Src0, init=C0),
        reference=lambda in0, s0: np.cumsum(in0.astype(np.float32), axis=-1) + s0,
    )
    shas = {}
    for ver in ("v3", "v4"):
        uops = lower(spec, ver=ver)
        shas[ver] = DveOpSpec(name="CUMSUM_ANT", opcode=1, uops=uops,
                              rd1_en=False).sha(ver)
    op = dve_ops.DveOp("CUMSUM_ANT", spec, subdim=False, uops_sha=shas)
    dve_ops.OPS.append(op)
    dve_ops.CUSTOM_DVE_SPECS["CUMSUM_ANT"] = spec
    dve_ops._SUB_OPCODE_FOR_NAME["CUMSUM_ANT"] = \
        max(dve_ops._SUB_OPCODE_FOR_NAME.values()) + 1
    return op


# ================================================================ host prep
def _prep(x, edge_index, batch_vec):
    src = np.asarray(edge_index[0], dtype=np.int64)
    dst = np.asarray(edge_index[1], dtype=np.int64)
    bv = np.asarray(batch_vec, dtype=np.int64)
    x16 = np.asarray(x, np.float32).astype(np.float16)

    gstart = np.searchsorted(bv, np.arange(N_GRAPHS))
    bounds = [0]
    for c in range(1, NC):
        target = c * (N_NODES // NC)
        gi = np.searchsorted(gstart, target)
        cand = []
        if gi < N_GRAPHS:
            cand.append(int(gstart[gi]))
        if gi > 0:
            cand.append(int(gstart[gi - 1]))
        bounds.append(min(cand, key=lambda v: abs(v - target)))
    bounds.append(N_NODES)
    n_lo = np.array(bounds[:-1])
    n_hi = np.array(bounds[1:])
    sizes = n_hi - n_lo
    assert sizes.max() <= NMAX, sizes
    g_lo = np.searchsorted(gstart, n_lo)
    g_hi = np.searchsorted(gstart, n_hi)

    owner = np.searchsorted(n_hi, dst, side='right')

    cores = []
    for c in range(NC):
        m = owner == c
        csrc = src[m]
        cdst_local = dst[m] - n_lo[c]
        size_c = int(sizes[c])

        xs = np.zeros((128, NCHUNK1 * ECH1), np.float16)
        gd1 = np.zeros((128, NCHUNK1 * NCH1 // 16), np.int16)
        ge2 = np.zeros((128, NCHUNK2 * ECH2 // 16), np.int16)
        gd2 = np.zeros((128, NCHUNK2 * NCH2 // 16), np.int16)

        blk_of = csrc >> 15
        src_local_all = (csrc & (BLK - 1))

        for k in range(NC):
            bm = blk_of == k
            bsrc = src_local_all[bm]
            bdst = cdst_local[bm]
            order = np.argsort(bdst, kind='stable')
            bsrc = bsrc[order]
            bdst = bdst[order]
            cnt = np.bincount(bdst, minlength=NMAX)
            cum = np.concatenate([[0], np.cumsum(cnt)])
            # x values of block k, feature-major [16, BLK]
            xbk = x16[BLK * k:BLK * (k + 1), :].T

            # L1: pre-gathered windowed stream (values instead of idx)
            for ch in range(NCHUNK1):
                a, b = ch * NCH1, (ch + 1) * NCH1
                e0, e1 = cum[a], cum[b]
                ne = int(e1 - e0)
                assert ne <= ECH1, (c, k, ch, ne, ECH1)
                vals = np.zeros((16, ECH1), np.float16)
                vals[:, :ne] = xbk[:, bsrc[e0:e1]]
                xs[16 * k:16 * (k + 1),
                   ch * ECH1:(ch + 1) * ECH1] = vals
                ends = (cum[a + 1:b + 1] - e0).astype(np.int16)
                _wrap_idx(ends, k, gd1, col0=ch * NCH1 // 16)

            # L2: index tensors (gathered on device from table2)
            for ch in range(NCHUNK2):
                a, b = ch * NCH2, (ch + 1) * NCH2
                e0, e1 = cum[a], cum[b]
                ne = int(e1 - e0)
                assert ne <= ECH2, (c, k, ch, ne, ECH2)
                ev = np.zeros(ECH2, np.int16)
                ev[:ne] = bsrc[e0:e1].astype(np.int16)
                _wrap_idx(ev, k, ge2, col0=ch * ECH2 // 16)
                ends = (cum[a + 1:b + 1] - e0).astype(np.int16)
                _wrap_idx(ends, k, gd2, col0=ch * NCH2 // 16)

        # POOL: mini-S selector per 128-node block (universal slot offsets)
        ngr = int(g_hi[c] - g_lo[c])
        assert ngr <= GW2, (c, ngr)
        bv_loc = bv[n_lo[c]:n_hi[c]] - g_lo[c]
        Sall = np.zeros((128, NBLKP * SW), np.float32)
        for b in range(NBLKP):
            a0 = b * 128
            a1 = min(a0 + 128, size_c)
            if a0 >= size_c:
                continue
            sl = bv_loc[a0:a1]
            o = _pool_off(b)
            assert int(sl.min()) >= o and int(sl.max()) < o + SW, \
                (c, b, o, sl.min(), sl.max())
            Sall[np.arange(a1 - a0), b * SW + (sl - o)] = 1.0

        cores.append(dict(
            n_lo=int(n_lo[c]), size=size_c, g_lo=int(g_lo[c]), ngr=ngr,
            xs=xs, gd1=gd1, ge2=ge2, gd2=gd2, Sall=Sall,
        ))
    return cores, [int(b) for b in bounds]


# ================================================================ device
def _build_program(bounds):
    from concourse import bacc, tile
    from concourse.bass import mybir

    CUMSUM = _register_cumsum()

    f32 = mybir.dt.float32
    f16 = mybir.dt.float16
    i16 = mybir.dt.int16
    RELU = mybir.ActivationFunctionType.Relu
    EXP = mybir.ActivationFunctionType.Exp
    SUB = mybir.AluOpType.subtract
    MUL = mybir.AluOpType.mult

    nc = bacc.Bacc("TRN2", target_bir_lowering=False, debug=False, num_devices=NC)

    def din(name, shape, dt):
        return nc.dram_tensor(name, shape, dt, kind="ExternalInput")

    xs_in = din("xs", [128, NCHUNK1 * ECH1], f16)
    xo_in = din("xo", [16, NMAX], f32)
    gd1_in = din("gd1", [128, NCHUNK1 * NCH1 // 16], i16)
    ge2_in = din("ge2", [128, NCHUNK2 * ECH2 // 16], i16)
    gd2_in = din("gd2", [128, NCHUNK2 * NCH2 // 16], i16)
    sall_in = din("sall", [128, NBLKP * SW], f32)
    w_ins = {}
    for nm, shape, dt in (
            ("w1e", [16, 16], f32), ("w1o", [16, 16], f32),
            ("b1e", [16, 1], f32), ("b1o", [16, 1], f32),
            ("w2e", [16, H], f32), ("w2o", [16, H], f32), ("b2", [H, 1], f32),
            ("gw1", [H, H], f32), ("gb1", [H, 1], f32),
            ("gw2", [H, H], f32), ("gb2", [H, 1], f32),
            ("gw3r", [H, H], f32), ("gb3c", [H, 1], f32),
            ("aw1", [H, H], f32), ("ab1", [H, 1], f32),
            ("aw2", [H, H], f32), ("ab2", [H, 1], f32),
            ("fw1", [H, H], f32), ("fb1", [H, 1], f32),
            ("fw2", [H, H], f32), ("fb2", [H, 1], f32),
            ("fw3r", [H, H], f32), ("fb3", [H, 1], f32),
            ("onesblk", [128, 16], f32), ("eye16", [16, 16], f32),
            ("eye16h", [16, 16], f16),
            ("eye3233", [H, 33], f32), ("e33", [1, 33], f32),
            ("ones132", [1, H], f32)):
        w_ins[nm] = din(nm, shape, dt)

    out_g = nc.dram_tensor("outg", [1, GW2], f32, kind="ExternalOutput")

    h1i_own = nc.dram_tensor("h1i_own", [16, NMAX, 2], f16)
    h1i_all = nc.dram_tensor("h1i_all", [NC * 16, NMAX, 2], f16, addr_space="Shared")
    h2_dram = nc.dram_tensor("h2d", [H, NMAX], f32)

    with tile.TileContext(nc) as tc:
        with (
            tc.tile_pool(name="sp", bufs=1) as sp,
            tc.tile_pool(name="wp", bufs=2) as wp,
            tc.tile_pool(name="wq", bufs=1) as wq,
            tc.tile_pool(name="pp", bufs=2, space="PSUM") as pp,
        ):
            W = {}
            for nm in ("w1e", "w1o", "b1e", "b1o", "w2e", "w2o", "b2",
                       "onesblk", "eye16", "eye16h"):
                t_in = w_ins[nm]
                W[nm] = sp.tile(list(t_in.shape), t_in.dtype, name=f"w_{nm}")
                nc.sync.dma_start(W[nm][:], t_in.ap()[:])

            # ---------------- Layer 1 (host-pregathered stream) ----------
            with tc.tile_pool(name="l1s", bufs=2) as l1p, nc.named_scope("L1"):
                for ch in range(NCHUNK1):
                    didx = wp.tile([128, NCH1 // 16], i16, tag="didx")
                    nc.sync.dma_start(
                        didx[:],
                        gd1_in.ap()[:, ch * NCH1 // 16:(ch + 1) * NCH1 // 16])
                    xoc = wq.tile([16, NCH1], f32, tag="xoc")
                    nc.sync.dma_start(
                        xoc[:], xo_in.ap()[:, ch * NCH1:(ch + 1) * NCH1])
                    stage1 = l1p.tile([128, ECH1], f16, tag="st1")
                    nc.sync.dma_start(
                        stage1[:], xs_in.ap()[:, ch * ECH1:(ch + 1) * ECH1])

                    cs = wq.tile([128, 1 + ECH1], f32, tag="cs")
                    nc.vector.memset(cs[:, 0:1], 0.0)
                    nc.vector._custom_dve(
                        CUMSUM, out=cs[:, 1:], in0=stage1[:], s0=0.0)

                    G = wq.tile([128, 1 + NCH1], f32, tag="G")
                    nc.vector.memset(G[:, 0:1], 0.0)
                    nc.gpsimd.ap_gather(
                        G[:, 1:], cs[:], didx[:],
                        channels=128, num_elems=1 + ECH1, d=1, num_idxs=NCH1)
                    P = wq.tile([128, NCH1], f32, tag="P")
                    nc.vector.tensor_tensor(P[:], G[:, 1:], G[:, :-1], SUB)

                    for t0 in range(0, NCH1, TILE_N):
                        tn = min(TILE_N, NCH1 - t0)
                        sl = slice(t0, t0 + tn)
                        pa = pp.tile([16, tn], f32, tag="pa")
                        nc.tensor.matmul(pa[:], W["onesblk"][:], P[:, sl],
                                         start=True, stop=False)
                        nc.tensor.matmul(pa[:], W["eye16"][:], xoc[:, sl],
                                         start=False, stop=True)
                        sa = wp.tile([16, tn], f32, tag="sa")
                        nc.vector.tensor_copy(sa[:], pa[:])
                        phe = pp.tile([16, tn], f32, tag="ph")
                        nc.tensor.matmul(phe[:], W["w1e"][:], sa[:],
                                         start=True, stop=True)
                        pho = pp.tile([16, tn], f32, tag="po")
                        nc.tensor.matmul(pho[:], W["w1o"][:], sa[:],
                                         start=True, stop=True)
                        he = wp.tile([16, tn, 2], f16, tag="he")
                        nc.scalar.activation(he[:, :, 0], phe[:], RELU,
                                             bias=W["b1e"][:])
                        nc.scalar.activation(he[:, :, 1], pho[:], RELU,
                                             bias=W["b1o"][:])
                        col = ch * NCH1 + t0
                        nc.sync.dma_start(
                            h1i_own.ap()[:, col:col + tn, :], he[:])

            # ---------------- exchange ----------------
            with nc.named_scope("AG"):
                nc.gpsimd.collective_compute(
                    "AllGather", mybir.AluOpType.bypass,
                    replica_groups=[list(range(NC))],
                    ins=[h1i_own.ap()[:]],
                    outs=[h1i_all.ap()[:]],
                )

            with tc.tile_pool(name="tbl", bufs=1) as tblp:
                # ---------------- table2 ----------------
                table2 = tblp.tile([128, BLK, 2], f16, tag="table")
                with nc.named_scope("T2"):
                    for k in range(NC):
                        lo, hi = k * BLK, (k + 1) * BLK
                        pos = lo
                        while pos < hi:
                            c2 = next(i for i in range(NC)
                                      if bounds[i] <= pos < bounds[i + 1])
                            seg_end = min(hi, bounds[c2 + 1])
                            ln = seg_end - pos
                            local = pos - bounds[c2]
                            nc.sync.dma_start(
                                table2[16 * k:16 * (k + 1),
                                       pos - lo:pos - lo + ln, :],
                                h1i_all.ap()[16 * c2:16 * (c2 + 1),
                                             local:local + ln, :])
                            pos = seg_end

                # ---------------- Layer 2 ----------------
                with nc.named_scope("L2"):
                    for ch in range(NCHUNK2):
                        gidx = wp.tile([128, ECH2 // 16], i16, tag="gidx")
                        nc.sync.dma_start(
                            gidx[:],
                            ge2_in.ap()[:, ch * ECH2 // 16:(ch + 1) * ECH2 // 16])
                        didx = wp.tile([128, NCH2 // 16], i16, tag="didx")
                        nc.sync.dma_start(
                            didx[:],
                            gd2_in.ap()[:, ch * NCH2 // 16:(ch + 1) * NCH2 // 16])
                        h1c = wq.tile([16, NCH2, 2], f16, tag="xoc")
                        nc.sync.dma_start(
                            h1c[:], h1i_own.ap()[:, ch * NCH2:(ch + 1) * NCH2, :])

                        stage = wq.tile([128, ECH2, 2], f16, tag="stage")
                        nc.gpsimd.ap_gather(
                            stage[:], table2[:], gidx[:],
                            channels=128, num_elems=BLK, d=2, num_idxs=ECH2)
                        cs2 = wq.tile([128, 1 + ECH2, 2], f32, tag="cs")
                        nc.vector.memset(cs2[:, 0:1, :], 0.0)
                        nc.vector._custom_dve(
                            CUMSUM, out=cs2[:, 1:, 0], in0=stage[:, :, 0], s0=0.0)
                        nc.vector._custom_dve(
                            CUMSUM, out=cs2[:, 1:, 1], in0=stage[:, :, 1], s0=0.0)

                        G2 = wq.tile([128, 1 + NCH2, 2], f32, tag="G")
                        nc.vector.memset(G2[:, 0:1, :], 0.0)
                        nc.gpsimd.ap_gather(
                            G2[:, 1:, :], cs2[:], didx[:],
                            channels=128, num_elems=1 + ECH2, d=2, num_idxs=NCH2)
                        P2 = wq.tile([128, NCH2, 2], f32, tag="P")
                        nc.vector.tensor_tensor(P2[:], G2[:, 1:, :], G2[:, :-1, :],
                                                SUB)

                        for t0 in range(0, NCH2, TILE_N):
                            tn = min(TILE_N, NCH2 - t0)
                            sl = slice(t0, t0 + tn)
                            pe = pp.tile([16, tn], f32, tag="pa")
                            nc.tensor.matmul(pe[:], W["onesblk"][:], P2[:, sl, 0],
                                             start=True, stop=False)
                            nc.tensor.matmul(pe[:], W["eye16h"][:], h1c[:, sl, 0],
                                             start=False, stop=True)
                            po = pp.tile([16, tn], f32, tag="po")
                            nc.tensor.matmul(po[:], W["onesblk"][:], P2[:, sl, 1],
                                             start=True, stop=False)
                            nc.tensor.matmul(po[:], W["eye16h"][:], h1c[:, sl, 1],
                                             start=False, stop=True)
                            se = wp.tile([16, tn], f32, tag="sa")
                            so = wp.tile([16, tn], f32, tag="so")
                            nc.vector.tensor_copy(se[:], pe[:])
                            nc.vector.tensor_copy(so[:], po[:])
                            ph2 = pp.tile([H, tn], f32, tag="ph")
                            nc.tensor.matmul(ph2[:], W["w2e"][:], se[:],
                                             start=True, stop=False)
                            nc.tensor.matmul(ph2[:], W["w2o"][:], so[:],
                                             start=False, stop=True)
                            h2t = wp.tile([H, tn], f32, tag="he")
                            nc.scalar.activation(h2t[:], ph2[:], RELU,
                                                 bias=W["b2"][:])
                            col = ch * NCH2 + t0
                            nc.sync.dma_start(
                                h2_dram.ap()[:, col:col + tn], h2t[:])

            # ---------------- pooling: exact one-hot PE segment sums -----
            with (
                tc.tile_pool(name="pool3", bufs=2) as p3,
                tc.tile_pool(name="pq", bufs=1, space="PSUM") as pq,
            ):
                for nm in ("gw1", "gb1", "gw2", "gb2", "gw3r", "gb3c",
                           "aw1", "ab1", "aw2", "ab2",
                           "fw1", "fb1", "fw2", "fb2", "fw3r", "fb3",
                           "eye3233", "e33", "ones132"):
                    t_in = w_ins[nm]
                    W[nm] = p3.tile(list(t_in.shape), t_in.dtype,
                                    name=f"w_{nm}", bufs=1)
                    nc.sync.dma_start(W[nm][:], t_in.ap()[:])
                Sall = p3.tile([128, NBLKP * SW], f32, bufs=1)
                nc.sync.dma_start(Sall[:], sall_in.ap()[:])

                psum_pool = pq.tile([33, GW2], f32)
                nc.vector.memset(psum_pool[:], 0.0)

                with nc.named_scope("POOL"):
                    for ti in range(NMAX // TILE_N):
                        t0 = ti * TILE_N
                        h2c = p3.tile([H, TILE_N], f32, tag="h2c")
                        nc.sync.dma_start(
                            h2c[:], h2_dram.ap()[:, t0:t0 + TILE_N])
                        pg = pp.tile([H, TILE_N], f32, tag="ph")
                        nc.tensor.matmul(pg[:], W["gw1"][:], h2c[:],
                                         start=True, stop=True)
                        g1 = p3.tile([H, TILE_N], f32, tag="g1")
                        nc.scalar.activation(g1[:], pg[:], RELU,
                                             bias=W["gb1"][:])
                        pg2 = pp.tile([H, TILE_N], f32, tag="ph")
                        nc.tensor.matmul(pg2[:], W["gw2"][:], g1[:],
                                         start=True, stop=True)
                        g2 = p3.tile([H, TILE_N], f32, tag="g2")
                        nc.scalar.activation(g2[:], pg2[:], RELU,
                                             bias=W["gb2"][:])
                        pg3 = pp.tile([H, TILE_N], f32, tag="ph")
                        nc.tensor.matmul(pg3[:], W["gw3r"][:], g2[:],
                                         start=True, stop=True)
                        ee = p3.tile([H, TILE_N], f32, tag="ee")
                        nc.scalar.activation(ee[:], pg3[:], EXP,
                                             bias=W["gb3c"][:])
                        pt = pp.tile([H, TILE_N], f32, tag="ph")
                        nc.tensor.matmul(pt[:], W["aw1"][:], h2c[:],
                                         start=True, stop=True)
                        t1 = p3.tile([H, TILE_N], f32, tag="g1")
                        nc.scalar.activation(t1[:], pt[:], RELU,
                                             bias=W["ab1"][:])
                        pt2 = pp.tile([H, TILE_N], f32, tag="ph")
                        nc.tensor.matmul(pt2[:], W["aw2"][:], t1[:],
                                         start=True, stop=True)
                        t2 = p3.tile([H, TILE_N], f32, tag="g2")
                        nc.scalar.activation(t2[:], pt2[:], RELU,
                                             bias=W["ab2"][:])
                        wt = p3.tile([H, TILE_N], f32, tag="wt")
                        nc.vector.tensor_tensor(wt[:], ee[:], t2[:], MUL)

                        for j in range(TILE_N // 128):
                            b = (t0 + j * 128) // 128
                            csl = slice(j * 128, (j + 1) * 128)
                            ptr = pp.tile([128, 33], f32, tag="pa")
                            nc.tensor.matmul(ptr[:], wt[:, csl], W["eye3233"][:],
                                             start=True, stop=False)
                            nc.tensor.matmul(ptr[:], ee[0:1, csl], W["e33"][:],
                                             start=False, stop=True)
                            wtT = p3.tile([128, 33], f32, tag="wtT")
                            nc.vector.tensor_copy(wtT[:], ptr[:])
                            o = _pool_off(b)
                            nc.tensor.matmul(
                                psum_pool[:, o:o + SW], wtT[:],
                                Sall[:, b * SW:(b + 1) * SW],
                                start=False, stop=True)

                    pooled = p3.tile([33, GW2], f32, bufs=1)
                    nc.vector.tensor_copy(pooled[:], psum_pool[:])
                    dclamp = p3.tile([1, GW2], f32, bufs=1)
                    nc.vector.tensor_scalar_max(
                        dclamp[:], pooled[32:33, :], 1e-30)
                    rec = p3.tile([1, GW2], f32, bufs=1)
                    nc.vector.reciprocal(rec[:], dclamp[:])
                    prb = pp.tile([H, GW2], f32, tag="ph")
                    nc.tensor.matmul(prb[:], W["ones132"][:], rec[:],
                                     start=True, stop=True)
                    recb = p3.tile([H, GW2], f32, bufs=1)
                    nc.vector.tensor_copy(recb[:], prb[:])
                    atth = p3.tile([H, GW2], f32, bufs=1)
                    nc.vector.tensor_tensor(atth[:], pooled[0:H, :], recb[:], MUL)

                    pf = pp.tile([H, GW2], f32, tag="ph")
                    nc.tensor.matmul(pf[:], W["fw1"][:], atth[:],
                                     start=True, stop=True)
                    o1 = p3.tile([H, GW2], f32, bufs=1)
                    nc.scalar.activation(o1[:], pf[:], RELU, bias=W["fb1"][:])
                    pf2 = pp.tile([H, GW2], f32, tag="ph")
                    nc.tensor.matmul(pf2[:], W["fw2"][:], o1[:],
                                     start=True, stop=True)
                    o2 = p3.tile([H, GW2], f32, bufs=1)
                    nc.scalar.activation(o2[:], pf2[:], RELU, bias=W["fb2"][:])
                    pf3 = pp.tile([H, GW2], f32, tag="ph")
                    nc.tensor.matmul(pf3[:], W["fw3r"][:], o2[:],
                                     start=True, stop=True)
                    o3 = p3.tile([H, GW2], f32, bufs=1)
                    nc.vector.tensor_scalar_add(o3[:], pf3[:], W["fb3"][:])
                    nc.sync.dma_start(out_g.ap()[:], o3[0:1, :])

    nc.compile()
    _split_multi_waits(nc, mybir)
    return nc


# ================================================================ entry
def kernel(x, w1, b1, w2, b2, gw1, gb1, gw2, gb2, gw3, gb3,
           aw1, ab1, aw2, ab2, fw1, fb1, fw2, fb2, fw3, fb3,
           edge_index, batch_vec, num_graphs):
    from concourse.bass_utils import run_bass_kernel_spmd

    x = np.asarray(x, np.float32)
    cores, bounds = _prep(x, edge_index, batch_vec)

    w1n = np.asarray(w1, np.float32)
    w1e_h = np.ascontiguousarray(w1n[:, 0::2])
    w1o_h = np.ascontiguousarray(w1n[:, 1::2])
    b1n = np.asarray(b1, np.float32)
    b1e_h = np.ascontiguousarray(b1n[0::2].reshape(16, 1))
    b1o_h = np.ascontiguousarray(b1n[1::2].reshape(16, 1))
    w2n = np.asarray(w2, np.float32)
    w2e = np.ascontiguousarray(w2n[0::2, :])
    w2o = np.ascontiguousarray(w2n[1::2, :])

    ones_blk = np.zeros((128, 16), np.float32)
    for p in range(128):
        ones_blk[p, p % 16] = 1.0
    eye16 = np.eye(16, dtype=np.float32)

    gw3r = np.tile(np.asarray(gw3, np.float32).reshape(H, 1), (1, H))
    fw3r = np.tile(np.asarray(fw3, np.float32).reshape(H, 1), (1, H))
    gb3c = np.full((H, 1),
                   float(np.asarray(gb3).reshape(-1)[0]) - SOFTMAX_SHIFT, np.float32)
    fb3c = np.full((H, 1), float(np.asarray(fb3).reshape(-1)[0]), np.float32)

    eye3233 = np.zeros((H, 33), np.float32)
    eye3233[:, :H] = np.eye(H, dtype=np.float32)
    e33 = np.zeros((1, 33), np.float32)
    e33[0, 32] = 1.0
    ones132 = np.ones((1, H), np.float32)

    def colb(a):
        return np.ascontiguousarray(np.asarray(a, np.float32).reshape(H, 1))

    common = dict(
        w1e=w1e_h, w1o=w1o_h, b1e=b1e_h, b1o=b1o_h,
        w2e=w2e, w2o=w2o, b2=colb(b2),
        gw1=np.asarray(gw1, np.float32), gb1=colb(gb1),
        gw2=np.asarray(gw2, np.float32), gb2=colb(gb2),
        gw3r=gw3r, gb3c=gb3c,
        aw1=np.asarray(aw1, np.float32), ab1=colb(ab1),
        aw2=np.asarray(aw2, np.float32), ab2=colb(ab2),
        fw1=np.asarray(fw1, np.float32), fb1=colb(fb1),
        fw2=np.asarray(fw2, np.float32), fb2=colb(fb2),
        fw3r=fw3r, fb3=fb3c,
        onesblk=ones_blk, eye16=eye16, eye16h=eye16.astype(np.float16),
        eye3233=eye3233, e33=e33, ones132=ones132,
    )

    in_maps = []
    for c, info in enumerate(cores):
        xo = np.zeros((16, NMAX), np.float32)
        xo[:, :info['size']] = x[info['n_lo']:info['n_lo'] + info['size'], :].T
        m = dict(common)
        m.update(xs=info['xs'], xo=xo, gd1=info['gd1'],
                 ge2=info['ge2'], gd2=info['gd2'], sall=info['Sall'])
        in_maps.append(m)

    key = tuple(bounds)
    if _cache.get('key') != key:
        _cache['nc'] = _build_program(bounds)
        _cache['key'] = key
    nc = _cache['nc']

    res = run_bass_kernel_spmd(nc, in_maps, core_ids=list(range(NC)),
                               trace=bool(os.environ.get("KERNEL_TRACE")))
    _cache['last_results'] = res

    out = np.zeros((N_GRAPHS, 1), np.float32)
    for c, info in enumerate(cores):
        vals = np.asarray(res.results[c]["outg"]).reshape(-1)
        out[info['g_lo']:info['g_lo'] + info['ngr'], 0] = vals[:info['ngr']]
    return out
